# revision 5
# baseline (speedup 1.0000x reference)
"""Self-contained Trainium2 Bass kernel for the 2-layer Llama3 model.

Sharding: token-sharded over 8 cores. Core c owns token blocks {c, 15-c}
(128 tokens each) of each batch -> 512 tokens/core, causally balanced.
Embeddings are gathered host-side and fed pre-transposed. Per layer each
core computes q/k/v for its own tokens, AllGathers k/v (bf16, 512KB/rank),
runs attention for its own query rows over the full sequence, then FFN and
the vocab head token-locally with full bf16 weights.

Device layouts: activations transposed [feature, token] in 128-partition
chunks; scores computed transposed [sk, sq]; softmax denominator via a
ones-augmented column on v; RoPE via a +-1 rotation matrix on the PE.
Weight fetches are coalesced into single large dma_start calls (k-blocked
3D access patterns) to keep DMA instruction count low.
"""
from contextlib import ExitStack

import numpy as np
import ml_dtypes

import concourse.bass as bass
from concourse.bacc import Bacc
import concourse.mybir as mybir
import concourse.tile as tile
from concourse.bass_utils import run_bass_kernel_spmd
from concourse.masks import make_identity

BF16 = ml_dtypes.bfloat16
F32 = mybir.dt.float32
BF = mybir.dt.bfloat16
I32 = mybir.dt.int32

V, D, H, KVH, HD, F, L, B, S = 32000, 1024, 16, 4, 64, 4096, 2, 2, 2048
ROPE_BASE = 500000.0
EPS = 1e-5
SCALE = 1.0 / HD ** 0.5
C = 8            # cores
NB = 16          # 128-token blocks per batch
BS = 128         # block size
OWN = 512        # own tokens per core (2 blocks x 2 batches)
NVT = (V + 511) // 512   # 63 head v-tiles (last is 256 wide)

Exp = mybir.ActivationFunctionType.Exp
Silu = mybir.ActivationFunctionType.Silu
Sqrt = mybir.ActivationFunctionType.Sqrt
Square = mybir.ActivationFunctionType.Square
MULT = mybir.AluOpType.mult
ADD = mybir.AluOpType.add


def own_blocks(c):
    return [c, NB - 1 - c]


def _colseg(b, t):
    """Own-token column range for (batch b, tile t in {0=low,1=high})."""
    return slice(256 * b + 128 * t, 256 * b + 128 * (t + 1))


def _kblk(ap, k, cols):
    """[128, nk*cols] SBUF tile -> block k column slice."""
    return ap[:, cols * k:cols * (k + 1)]


# ---------------------------------------------------------------- device ---

def build_nc():
    nc = Bacc()

    xin = nc.dram_tensor("xin", [128, 8 * OWN], F32, kind="ExternalInput")
    cosT = nc.dram_tensor("cosT", [128, OWN], F32, kind="ExternalInput")
    sinT = nc.dram_tensor("sinT", [128, OWN], F32, kind="ExternalInput")
    rmat = nc.dram_tensor("rmat", [128, 128], BF, kind="ExternalInput")
    masks = nc.dram_tensor("masks", [16, 128, 512], BF, kind="ExternalInput")
    wq = nc.dram_tensor("wq", [L, D, D], BF, kind="ExternalInput")
    wk = nc.dram_tensor("wk", [L, D, KVH * HD], BF, kind="ExternalInput")
    wv = nc.dram_tensor("wv", [L, D, KVH * HD], BF, kind="ExternalInput")
    wo = nc.dram_tensor("wo", [L, D, D], BF, kind="ExternalInput")
    wg = nc.dram_tensor("wg", [L, D, F], BF, kind="ExternalInput")
    wu = nc.dram_tensor("wu", [L, D, F], BF, kind="ExternalInput")
    wd = nc.dram_tensor("wd", [L, F, D], BF, kind="ExternalInput")
    wout = nc.dram_tensor("wout", [D, V], BF, kind="ExternalInput")
    logits = nc.dram_tensor("logits", [OWN, V], F32, kind="ExternalOutput")

    kvs = [nc.dram_tensor(f"kvs{l}", [B, 2, 256, 256], BF) for l in range(L)]
    kvr = [nc.dram_tensor(f"kvr{l}", [C, B, 2, 256, 256], BF,
                          addr_space="Shared") for l in range(L)]

    # 3D views of the big weights: [k, 128, cols]
    wq_v = [wq[l].rearrange("(k p) c -> k p c", p=128) for l in range(L)]
    wk_v = [wk[l].rearrange("(k p) c -> k p c", p=128) for l in range(L)]
    wv_v = [wv[l].rearrange("(k p) c -> k p c", p=128) for l in range(L)]
    wo_v = [wo[l].rearrange("(k p) c -> k p c", p=128) for l in range(L)]
    wg_v = [wg[l].rearrange("(k p) c -> k p c", p=128) for l in range(L)]
    wu_v = [wu[l].rearrange("(k p) c -> k p c", p=128) for l in range(L)]
    wd_v = [wd[l].rearrange("(k p) c -> k p c", p=128) for l in range(L)]
    wout_v = wout.rearrange("(k p) c -> k p c", p=128)

    with tile.TileContext(nc) as tc, ExitStack() as st:
        const = st.enter_context(tc.tile_pool(name="const", bufs=1))
        resid = st.enter_context(tc.tile_pool(name="resid", bufs=1))
        npool = st.enter_context(tc.tile_pool(name="npool", bufs=1))
        qpool = st.enter_context(tc.tile_pool(name="qpool", bufs=1))
        apool = st.enter_context(tc.tile_pool(name="apool", bufs=1))
        hpool = st.enter_context(tc.tile_pool(name="hpool", bufs=1))
        sb = st.enter_context(tc.tile_pool(name="sb", bufs=2))
        wbig = st.enter_context(tc.tile_pool(name="wbig", bufs=2))
        psA = st.enter_context(tc.tile_pool(name="psA", bufs=2, space="PSUM"))
        psB = st.enter_context(tc.tile_pool(name="psB", bufs=2, space="PSUM"))
        psM = st.enter_context(tc.tile_pool(name="psM", bufs=4, space="PSUM"))

        # constants
        ident = const.tile([128, 128], F32, tag="ident")
        make_identity(nc, ident)
        ones_col = const.tile([128, 1], BF, tag="ones_col")
        nc.any.memset(ones_col[:], 1.0)
        ones_row = const.tile([1, 128], BF, tag="ones_row")
        nc.any.memset(ones_row[:], 1.0)
        eps_t = const.tile([1, 1], F32, tag="eps")
        nc.any.memset(eps_t[:], EPS)
        t_rmat = const.tile([128, 128], BF, tag="rmat")
        nc.sync.dma_start(out=t_rmat[:], in_=rmat[:])
        t_cos = const.tile([128, OWN], F32, tag="cos")
        nc.sync.dma_start(out=t_cos[:], in_=cosT[:])
        t_sin = const.tile([128, OWN], F32, tag="sin")
        nc.sync.dma_start(out=t_sin[:], in_=sinT[:])
        t_masks = const.tile([128, 16 * 512], BF, tag="masks")
        nc.sync.dma_start(
            out=t_masks[:].rearrange("p (m c) -> p m c", m=16),
            in_=masks[:].transpose([1, 0, 2]))

        def mask_m(m):
            return t_masks[:, 512 * m:512 * (m + 1)]

        # residual stream xT: [128, 8*OWN] f32, chunk k = cols 512k..
        x = resid.tile([128, 8 * OWN], F32, tag="x", name="x")
        nc.sync.dma_start(out=x[:], in_=xin[:])

        def xk(k):
            return x[:, OWN * k:OWN * (k + 1)]

        def rmsnorm():
            """x -> n bf16 [128, 8*OWN] (chunk k = cols 512k..)."""
            ssq = psB.tile([1, OWN], F32, tag="psB")
            for k in range(8):
                x2 = sb.tile([128, OWN], BF, tag="x2")
                nc.scalar.activation(out=x2[:], in_=xk(k), func=Square)
                nc.tensor.matmul(out=ssq[:], lhsT=ones_col[:], rhs=x2[:],
                                 start=(k == 0), stop=(k == 7))
            rms = sb.tile([1, OWN], F32, tag="rms", bufs=1)
            nc.scalar.activation(out=rms[:], in_=ssq[:], func=Sqrt,
                                 scale=1.0 / D, bias=eps_t[:])
            inv = sb.tile([1, OWN], F32, tag="inv", bufs=1)
            nc.vector.reciprocal(out=inv[:], in_=rms[:])
            inv_bf = sb.tile([1, OWN], BF, tag="invbf", bufs=1)
            nc.vector.tensor_copy(out=inv_bf[:], in_=inv[:])
            binv = psB.tile([128, OWN], F32, tag="psB")
            nc.tensor.matmul(out=binv[:], lhsT=ones_row[:], rhs=inv_bf[:],
                             start=True, stop=True)
            n = npool.tile([128, 8 * OWN], BF, tag="n", name="n")
            for k in range(8):
                nc.vector.tensor_tensor(out=_kblk(n, k, OWN), in0=xk(k),
                                        in1=binv[:], op=MULT)
            return n

        def rope(pm, dst):
            """pm: psum [128, OWN] pre-rope -> bf16 dst slice with rope."""
            yr = sb.tile([128, OWN], BF, tag="prerope")
            nc.vector.tensor_copy(out=yr[:], in_=pm[:])
            rot = psA.tile([128, OWN], F32, tag="psA")
            nc.tensor.matmul(out=rot[:], lhsT=t_rmat[:], rhs=yr[:],
                             start=True, stop=True)
            tmp1 = sb.tile([128, OWN], F32, tag="ropet1", bufs=1)
            nc.vector.tensor_tensor(out=tmp1[:], in0=yr[:], in1=t_cos[:], op=MULT)
            tmp2 = sb.tile([128, OWN], F32, tag="ropet2", bufs=1)
            nc.vector.tensor_tensor(out=tmp2[:], in0=rot[:], in1=t_sin[:], op=MULT)
            nc.vector.tensor_tensor(out=dst, in0=tmp1[:], in1=tmp2[:], op=ADD)

        for l in range(L):
            n = rmsnorm()

            # ---- q projection: out qr chunks [128, OWN], rope applied ----
            qr = qpool.tile([128, 8 * OWN], BF, tag="qr", name="qr")
            for mb in range(2):
                wt = wbig.tile([128, 8 * 512], BF, tag="w4k", name="wqt")
                nc.sync.dma_start(
                    out=wt[:].rearrange("p (k c) -> p k c", k=8),
                    in_=wq_v[l][:, :, 512 * mb:512 * (mb + 1)]
                        .transpose([1, 0, 2]))
                for ms in range(4):
                    mo = 4 * mb + ms
                    pm = psM.tile([128, OWN], F32, tag="pmm")
                    for k in range(8):
                        nc.tensor.matmul(
                            out=pm[:], lhsT=wt[:, 512 * k + 128 * ms:
                                               512 * k + 128 * (ms + 1)],
                            rhs=_kblk(n, k, OWN), start=(k == 0), stop=(k == 7))
                    rope(pm, _kblk(qr, mo, OWN))

            # ---- k projection (2 chunks of 128 rows) + rope ----
            kr = sb.tile([128, 2 * OWN], BF, tag="kr", name="kr")
            wtk = wbig.tile([128, 8 * 256], BF, tag="wkv", name="wkt")
            nc.sync.dma_start(
                out=wtk[:].rearrange("p (k c) -> p k c", k=8),
                in_=wk_v[l][:].transpose([1, 0, 2]))
            for mo in range(2):
                pm = psM.tile([128, OWN], F32, tag="pmm")
                for k in range(8):
                    nc.tensor.matmul(
                        out=pm[:], lhsT=wtk[:, 256 * k + 128 * mo:
                                            256 * k + 128 * (mo + 1)],
                        rhs=_kblk(n, k, OWN), start=(k == 0), stop=(k == 7))
                rope(pm, _kblk(kr, mo, OWN))

            # ---- v natural [own tok, 256]; lhsT = n col-slices ----
            wtv = wbig.tile([128, 8 * 256], BF, tag="wkv", name="wvt")
            nc.sync.dma_start(
                out=wtv[:].rearrange("p (k c) -> p k c", k=8),
                in_=wv_v[l][:].transpose([1, 0, 2]))
            for t in range(4):
                pv = psM.tile([128, 256], F32, tag="pmm")
                for k in range(8):
                    nc.tensor.matmul(
                        out=pv[:],
                        lhsT=n[:, OWN * k + 128 * t:OWN * k + 128 * (t + 1)],
                        rhs=wtv[:, 256 * k:256 * (k + 1)],
                        start=(k == 0), stop=(k == 7))
                vt = sb.tile([128, 256], BF, tag="vnat")
                nc.vector.tensor_copy(out=vt[:], in_=pv[:])
                nc.sync.dma_start(
                    out=kvs[l][t // 2, 1, 128 * (t % 2):128 * (t % 2 + 1), :],
                    in_=vt[:])
            for b in range(B):
                for mo in range(2):
                    nc.sync.dma_start(
                        out=kvs[l][b, 0, 128 * mo:128 * (mo + 1), :],
                        in_=kr[:, OWN * mo + 256 * b:OWN * mo + 256 * (b + 1)])
            nc.gpsimd.collective_compute(
                "AllGather", mybir.AluOpType.bypass,
                replica_groups=[list(range(C))],
                ins=[kvs[l][:]], outs=[kvr[l][:]])

            # ---- attention (per batch: assemble k/v, run units) ----
            casm = apool.tile([128, 8 * OWN], BF, tag="casm", name="casm")
            for b in range(B):
                kT = [apool.tile([64, S], BF, tag=f"kt{g}", name=f"kt{g}")
                      for g in range(KVH)]
                for g in range(KVH):
                    src = kvr[l][:, b, 0, 64 * g:64 * (g + 1), :]   # [C,64,256]
                    # low blocks of ranks 0..7 -> cols 0:1024, one DMA
                    nc.sync.dma_start(
                        out=kT[g][:, 0:1024].rearrange("p (r c) -> p r c", r=C),
                        in_=src[:, :, 0:128].transpose([1, 0, 2]))
                    # high block of rank r -> block 15-r
                    for r in range(C):
                        nc.sync.dma_start(
                            out=kT[g][:, 128 * (NB - 1 - r):128 * (NB - r)],
                            in_=src[r, :, 128:256])
                v4 = [apool.tile([128, 260], BF, tag=f"v4{j}", name=f"v4{j}")
                      for j in range(NB)]
                for j in range(NB):
                    r, i = (j, 0) if j < C else (NB - 1 - j, 1)
                    dst = v4[j][:].rearrange("p (g c) -> p g c", g=4)
                    nc.sync.dma_start(
                        out=dst[:, :, 0:64],
                        in_=kvr[l][r, b, 1, 128 * i:128 * (i + 1), :]
                            .rearrange("p (g c) -> p g c", g=4))
                    nc.any.memset(dst[:, :, 64:65], 1.0)

                for g in range(KVH):
                    for t in range(2):          # t=0: low block, t=1: high
                        qp = sb.tile([64, 512], BF, tag="qpack")
                        for i in range(4):
                            h = 4 * g + i
                            mo, ro = divmod(h, 2)
                            nc.vector.tensor_copy(
                                out=qp[:, 128 * i:128 * (i + 1)],
                                in_=qr[64 * ro:64 * (ro + 1),
                                       OWN * mo + 256 * b + 128 * t:
                                       OWN * mo + 256 * b + 128 * (t + 1)])
                        ctx = psB.tile([65, 512], F32, tag="psB")
                        nj = 8 if t == 0 else 16
                        for j in range(nj):
                            sc = psA.tile([128, 512], F32, tag="psA")
                            nc.tensor.matmul(
                                out=sc[:], lhsT=kT[g][:, 128 * j:128 * (j + 1)],
                                rhs=qp[:], start=True, stop=True)
                            ex = sb.tile([128, 512], BF, tag="exp")
                            nc.scalar.activation(out=ex[:], in_=sc[:], func=Exp)
                            if t == 0 or j >= 8:
                                exm = sb.tile([128, 512], BF, tag="expm")
                                m = mask_m(j if t == 0 else j)
                                nc.vector.tensor_tensor(out=exm[:], in0=ex[:],
                                                        in1=m, op=MULT)
                                ex = exm
                            nc.tensor.matmul(
                                out=ctx[:], lhsT=v4[j][:, 65 * g:65 * (g + 1)],
                                rhs=ex[:], start=(j == 0), stop=(j == nj - 1))
                        rec = sb.tile([1, 512], F32, tag="rec")
                        nc.vector.reciprocal(out=rec[:], in_=ctx[64:65, :])
                        rec_bf = sb.tile([1, 512], BF, tag="recbf")
                        nc.vector.tensor_copy(out=rec_bf[:], in_=rec[:])
                        brec = psA.tile([64, 512], F32, tag="psA")
                        nc.tensor.matmul(out=brec[:], lhsT=ones_row[:1, 0:64],
                                         rhs=rec_bf[:], start=True, stop=True)
                        brec_s = sb.tile([64, 512], BF, tag="brecs")
                        nc.vector.tensor_copy(out=brec_s[:], in_=brec[:])
                        for i in range(4):
                            h = 4 * g + i
                            mo, ro = divmod(h, 2)
                            nc.vector.tensor_tensor(
                                out=casm[64 * ro:64 * (ro + 1),
                                         OWN * mo + 256 * b + 128 * t:
                                         OWN * mo + 256 * b + 128 * (t + 1)],
                                in0=ctx[0:64, 128 * i:128 * (i + 1)],
                                in1=brec_s[:, 128 * i:128 * (i + 1)], op=MULT)

            # ---- wo + residual ----
            for mb in range(2):
                wt = wbig.tile([128, 8 * 512], BF, tag="w4k", name="wot")
                nc.sync.dma_start(
                    out=wt[:].rearrange("p (k c) -> p k c", k=8),
                    in_=wo_v[l][:, :, 512 * mb:512 * (mb + 1)]
                        .transpose([1, 0, 2]))
                for ms in range(4):
                    mo = 4 * mb + ms
                    pm = psM.tile([128, OWN], F32, tag="pmm")
                    for k in range(8):
                        nc.tensor.matmul(
                            out=pm[:], lhsT=wt[:, 512 * k + 128 * ms:
                                               512 * k + 128 * (ms + 1)],
                            rhs=_kblk(casm, k, OWN), start=(k == 0), stop=(k == 7))
                    nc.vector.tensor_tensor(out=xk(mo), in0=xk(mo),
                                            in1=pm[:], op=ADD)

            # ---- FFN ----
            n2 = rmsnorm()
            ht = hpool.tile([128, 32 * OWN], BF, tag="ht", name="ht")
            for mb in range(8):
                wgt_h = []
                wut_h = []
                for hf in range(2):
                    a = wbig.tile([128, 4 * 512], BF, tag="wga", name="wgt")
                    nc.sync.dma_start(
                        out=a[:].rearrange("p (k c) -> p k c", k=4),
                        in_=wg_v[l][4 * hf:4 * (hf + 1),
                                    :, 512 * mb:512 * (mb + 1)]
                            .transpose([1, 0, 2]))
                    wgt_h.append(a)
                    u = wbig.tile([128, 4 * 512], BF, tag="wua", name="wut")
                    nc.sync.dma_start(
                        out=u[:].rearrange("p (k c) -> p k c", k=4),
                        in_=wu_v[l][4 * hf:4 * (hf + 1),
                                    :, 512 * mb:512 * (mb + 1)]
                            .transpose([1, 0, 2]))
                    wut_h.append(u)
                for ms in range(4):
                    mo = 4 * mb + ms
                    pg = psM.tile([128, OWN], F32, tag="pmm")
                    for k in range(8):
                        nc.tensor.matmul(
                            out=pg[:],
                            lhsT=wgt_h[k // 4][:, 512 * (k % 4) + 128 * ms:
                                               512 * (k % 4) + 128 * (ms + 1)],
                            rhs=_kblk(n2, k, OWN), start=(k == 0), stop=(k == 7))
                    gs = sb.tile([128, OWN], BF, tag="gsilu")
                    nc.scalar.activation(out=gs[:], in_=pg[:], func=Silu)
                    pu = psM.tile([128, OWN], F32, tag="pmm")
                    for k in range(8):
                        nc.tensor.matmul(
                            out=pu[:],
                            lhsT=wut_h[k // 4][:, 512 * (k % 4) + 128 * ms:
                                               512 * (k % 4) + 128 * (ms + 1)],
                            rhs=_kblk(n2, k, OWN), start=(k == 0), stop=(k == 7))
                    nc.vector.tensor_tensor(out=_kblk(ht, mo, OWN), in0=pu[:],
                                            in1=gs[:], op=MULT)

            # ---- FFN down: 2 groups of 4 output chunks, stream wd quarters
            for grp in range(2):
                pd = [psM.tile([128, OWN], F32, tag="pmm", name=f"pd{m}")
                      for m in range(4)]
                for qb in range(8):
                    wdt = wbig.tile([128, 4 * 1024], BF, tag="w4k", name="wdt")
                    nc.sync.dma_start(
                        out=wdt[:].rearrange("p (k c) -> p k c", k=4),
                        in_=wd_v[l][4 * qb:4 * (qb + 1)]
                            .transpose([1, 0, 2]))
                    for k in range(4):
                        kg = 4 * qb + k
                        for m in range(4):
                            mo = 4 * grp + m
                            nc.tensor.matmul(
                                out=pd[m][:],
                                lhsT=wdt[:, 1024 * k + 128 * mo:
                                         1024 * k + 128 * (mo + 1)],
                                rhs=_kblk(ht, kg, OWN),
                                start=(kg == 0), stop=(kg == 31))
                for m in range(4):
                    mo = 4 * grp + m
                    nc.vector.tensor_tensor(out=xk(mo), in0=xk(mo),
                                            in1=pd[m][:], op=ADD)

        # ---- final norm + head ----
        nf = rmsnorm()
        for vt in range(NVT):
            vw = min(512, V - 512 * vt)
            wt = wbig.tile([128, 8 * 512], BF, tag="w4k", name="wht")
            nc.sync.dma_start(
                out=wt[:, :8 * vw].rearrange("p (k c) -> p k c", k=8),
                in_=wout_v[:, :, 512 * vt:512 * vt + vw]
                    .transpose([1, 0, 2]))
            for t in range(4):
                ph = psM.tile([128, 512], F32, tag="pmm")
                for k in range(8):
                    nc.tensor.matmul(
                        out=ph[:, :vw],
                        lhsT=nf[:, OWN * k + 128 * t:OWN * k + 128 * (t + 1)],
                        rhs=wt[:, vw * k:vw * (k + 1)],
                        start=(k == 0), stop=(k == 7))
                ot = sb.tile([128, 512], F32, tag="hout")
                nc.vector.tensor_copy(out=ot[:, :vw], in_=ph[:, :vw])
                nc.sync.dma_start(
                    out=logits[128 * t:128 * (t + 1), 512 * vt:512 * vt + vw],
                    in_=ot[:, :vw])

    return nc


# ------------------------------------------------------------------ host ---

_NC_CACHE = {}


def _get_nc():
    if "nc" not in _NC_CACHE:
        nc = build_nc()
        nc.finalize()
        _NC_CACHE["nc"] = nc
    return _NC_CACHE["nc"]


def _host_prep(inputs):
    inv_freq = 1.0 / ROPE_BASE ** (np.arange(0, HD, 2, dtype=np.float32) / HD)
    t = np.arange(S, dtype=np.float32)
    freqs = t[:, None] * inv_freq[None, :]
    ang = np.concatenate([freqs, freqs], axis=-1)       # [S, 64]
    cos_full, sin_full = np.cos(ang), np.sin(ang)
    cosT2 = np.empty((128, S), np.float32)
    sinT2 = np.empty((128, S), np.float32)
    for p in range(128):
        d = p % 64
        cosT2[p] = cos_full[:, d]
        sinT2[p] = sin_full[:, d] * (-1.0 if d < 32 else 1.0)

    R = np.zeros((128, 128), np.float32)
    for blk in range(2):
        o = blk * 64
        for j in range(32):
            R[o + 32 + j, o + j] = 1.0
            R[o + j, o + 32 + j] = 1.0

    naw = np.asarray(inputs["norm_attn_w"], np.float32)
    nfw = np.asarray(inputs["norm_ff_w"], np.float32)
    emb = np.asarray(inputs["token_emb"], np.float32)
    prep = {
        "rmat": np.ascontiguousarray(R.astype(BF16)),
        "wq": np.ascontiguousarray(
            (np.asarray(inputs["wq"], np.float32) * naw[:, :, None] * SCALE).astype(BF16)),
        "wk": np.ascontiguousarray(
            (np.asarray(inputs["wk"], np.float32) * naw[:, :, None]).astype(BF16)),
        "wv": np.ascontiguousarray(
            (np.asarray(inputs["wv"], np.float32) * naw[:, :, None]).astype(BF16)),
        "wo": np.ascontiguousarray(np.asarray(inputs["wo"], np.float32).astype(BF16)),
        "wg": np.ascontiguousarray(
            (np.asarray(inputs["w_gate"], np.float32) * nfw[:, :, None]).astype(BF16)),
        "wu": np.ascontiguousarray(
            (np.asarray(inputs["w_up"], np.float32) * nfw[:, :, None]).astype(BF16)),
        "wd": np.ascontiguousarray(np.asarray(inputs["w_down"], np.float32).astype(BF16)),
        "wout": np.ascontiguousarray(
            (np.asarray(inputs["w_out"], np.float32)
             * np.asarray(inputs["norm_final_w"], np.float32)[:, None]).astype(BF16)),
    }

    idx_full = np.asarray(inputs["in_idx"], np.int64)
    tri = (np.arange(128)[:, None] <= np.arange(128)[None, :]).astype(np.float32)
    tri4 = np.tile(tri, (1, 4))
    in_maps = []
    for c in range(C):
        blks = own_blocks(c)
        pos = np.concatenate([np.arange(bl * BS, (bl + 1) * BS) for bl in blks])
        # own-token embeddings, transposed, k-chunk layout [128, 8*512]
        own_tok = np.concatenate(
            [idx_full[b, blks[t] * BS:(blks[t] + 1) * BS]
             for b in range(B) for t in range(2)])
        xg = emb[own_tok]                       # [512, 1024]
        xT = np.ascontiguousarray(xg.T)         # [1024, 512]
        xin = np.ascontiguousarray(
            xT.reshape(8, 128, OWN).transpose(1, 0, 2).reshape(128, 8 * OWN))
        cosT = np.ascontiguousarray(
            np.concatenate([cosT2[:, pos], cosT2[:, pos]], axis=1))
        sinT = np.ascontiguousarray(
            np.concatenate([sinT2[:, pos], sinT2[:, pos]], axis=1))
        # masks [16, 128, 512]: 0..7 low block (blk c) j=0..7,
        #                       8..15 high block (blk 15-c) j=8..15
        mk = np.zeros((16, 128, 512), np.float32)
        for t, blk in enumerate(blks):
            for jj in range(8):
                j = jj if t == 0 else jj + 8
                if j < blk:
                    mk[8 * t + jj] = 1.0
                elif j == blk:
                    mk[8 * t + jj] = tri4
        in_maps.append({
            "xin": xin,
            "cosT": cosT,
            "sinT": sinT,
            "masks": np.ascontiguousarray(mk.astype(BF16)),
            **prep,
        })
    return in_maps


def _assemble(results):
    out = np.empty((B, S, V), np.float32)
    for c in range(C):
        lg = np.asarray(results[c]["logits"])
        blks = own_blocks(c)
        for b in range(B):
            for t, blk in enumerate(blks):
                out[b, blk * BS:(blk + 1) * BS] = \
                    lg[256 * b + 128 * t:256 * b + 128 * (t + 1)]
    return out


def run(inputs, trace=False):
    nc = _get_nc()
    in_maps = _host_prep(inputs)
    res = run_bass_kernel_spmd(nc, in_maps, list(range(C)), trace=trace)
    return _assemble(res.results), res


def kernel(**inputs):
    out, _ = run(inputs)
    return out
